# revision 20
# baseline (speedup 1.0000x reference)
# Trainium2 Bass kernel for nn_DecoderMHA (dense decoder multi-head attention).
#
# Sharding (8 NeuronCores): batch (4) x tensor-parallel over heads (2).
# Core c handles batch b = c//2 and heads [tp*8, tp*8+8) where tp = c%2,
# i.e. a 512-wide slice of the QKV projection output dim and the matching
# 512 rows of Wo^T. Per-core partial outputs are summed on the host
# (y[b] = part[b,0] + part[b,1] + bo).
#
# Per-core pipeline (matmul operands bf16, fp32 PSUM accumulation), built
# around PE-array tile packing (tile_position auto-derived from the APs'
# base partitions):
#   A) Q^T/K^T [512,2048] and V [2048,512] projections from x^T, emitted
#      e-chunk by e-chunk and interleaved with attention so the Tile
#      scheduler can overlap phases (each phase has its own PSUM ring).
#   B) Attention runs per head-PAIR (heads 2t, 2t+1 share qT[t]/kT[t] at
#      partition rows 0-63 / 64-127):
#        - scores^T for A and B land in one [128,1024] PSUM strip-pair;
#          the two K=64 matmuls occupy disjoint PE row-groups and run
#          concurrently.
#        - one ScalarE activation computes exp(0.125*s + pad_bias) for
#          both halves; causal-mask multiply on diagonal blocks only.
#        - attn@V for A and B write rows 0-63 / 64-127 of one [128,512]
#          PSUM bank (disjoint PE col-groups -> concurrent).
#        - softmax denominators come from M=1 ones-matmuls into rows 0/64
#          of a second bank (also concurrent, accumulated over kc).
#      Unnormalised out^T goes straight to SBUF bf16; denominators are
#      staged to DRAM, read back [4,512]-batched per pair, reciprocated,
#      and broadcast back for one in-place normalise multiply per pair.
#   C) y = out @ Wo^T partial via out^T-as-lhsT matmuls, bf16 output DMA.
import os
import numpy as np

BSZ, SEQ, DM = 4, 2048, 1024
HEADS, DK = 16, 64
NCORES, TP = 8, 2
E = DM // TP          # 512 per-core projection slice
HPC = HEADS // TP     # 8 heads per core
P = 128
NDC = DM // P         # 8 contraction chunks
NEC = E // P          # 4 e-chunks (head pairs)
NSC = SEQ // P        # 16 sequence chunks
NSB = SEQ // 512      # 4 sequence blocks
NQB = 4               # q blocks of 512
SCALE = 1.0 / float(np.sqrt(DK))

_CACHED = {}


def _split_sync_waits(nc, mybir, max_waits=1):
    """The walrus in this container only accepts one sync-wait per
    instruction; move excess waits onto NoOps in front."""
    n = 0
    for fn in nc.m.functions:
        for bb in fn.blocks:
            insts = bb.instructions
            i = 0
            while i < len(insts):
                inst = insts[i]
                si = getattr(inst, "sync_info", None)
                if si is not None and si.on_wait and len(si.on_wait) > max_waits:
                    waits = list(si.on_wait)
                    extra, keep = waits[:-max_waits], waits[-max_waits:]
                    si.on_wait = keep
                    pos = i
                    for j in range(0, len(extra), max_waits):
                        nop = mybir.InstNoOp(
                            name=nc.get_next_instruction_name(),
                            sync_info=mybir.SyncInfo(
                                on_wait=extra[j:j + max_waits], on_update=[]),
                            bass_nofuse=True,
                            engine=inst.engine,
                        )
                        insts.insert(pos, nop)
                        pos += 1
                        i += 1
                        n += 1
                i += 1
    return n


def _build():
    import concourse.bass as bass
    from concourse import mybir
    from concourse.tile import TileContext

    f32 = mybir.dt.float32
    bf16 = mybir.dt.bfloat16
    Exp = mybir.ActivationFunctionType.Exp
    MUL = mybir.AluOpType.mult
    ADD = mybir.AluOpType.add

    nc = bass.Bass("TRN2", target_bir_lowering=False, debug=False,
                   num_devices=NCORES)

    # DRAM I/O (per-core layouts, pre-tiled on host)
    xt = nc.dram_tensor("xt", [NSB, P, NDC, 512], bf16, kind="ExternalInput")
    wq = nc.dram_tensor("wq", [P, NDC, E], bf16, kind="ExternalInput")
    wk = nc.dram_tensor("wk", [P, NDC, E], bf16, kind="ExternalInput")
    wv = nc.dram_tensor("wv", [P, NDC, E], bf16, kind="ExternalInput")
    wo = nc.dram_tensor("wo", [P, NEC, DM], bf16, kind="ExternalInput")
    bqt = nc.dram_tensor("bqt", [P, NEC], f32, kind="ExternalInput")
    bkt = nc.dram_tensor("bkt", [P, NEC], f32, kind="ExternalInput")
    bvb = nc.dram_tensor("bvb", [P, E], f32, kind="ExternalInput")
    cm = nc.dram_tensor("cm", [P, P], f32, kind="ExternalInput")
    pb = nc.dram_tensor("pb", [P, NSC], f32, kind="ExternalInput")
    ones = nc.dram_tensor("ones", [P, 1], bf16, kind="ExternalInput")
    y = nc.dram_tensor("y", [SEQ, DM], bf16, kind="ExternalOutput")

    with TileContext(nc) as tc:
        with (
            tc.tile_pool(name="persist", bufs=1) as pp,
            tc.tile_pool(name="work", bufs=2) as pw,
            tc.tile_pool(name="psA", bufs=2, space="PSUM") as psA,
            tc.tile_pool(name="psS", bufs=2, space="PSUM") as psS,
            tc.tile_pool(name="psO", bufs=1, space="PSUM") as psO,
            tc.tile_pool(name="psD", bufs=1, space="PSUM") as psD,
            tc.tile_pool(name="scr", bufs=2, space="DRAM") as scr,
        ):
            # ---- persistent SBUF ----
            qT = [pp.tile([P, SEQ], bf16, tag=f"qT{t}", name=f"qT{t}")
                  for t in range(NEC)]
            kT = [pp.tile([P, SEQ], bf16, tag=f"kT{t}", name=f"kT{t}")
                  for t in range(NEC)]
            vS = [pp.tile([P, HPC, DK], bf16, tag=f"vS{g}", name=f"vS{g}")
                  for g in range(NSC)]
            outT = [pp.tile([P, SEQ], bf16, tag=f"oT{t}", name=f"oT{t}")
                    for t in range(NEC)]
            cm_s = pp.tile([P, P], f32, tag="cm")
            pb_s = pp.tile([P, NSC], f32, tag="pb")
            bq_s = pp.tile([P, NEC], f32, tag="bq")
            bk_s = pp.tile([P, NEC], f32, tag="bk")
            bv_s = pp.tile([P, E], f32, tag="bv")
            ones_s = pp.tile([P, 1], bf16, tag="ones")

            xt_s = [pp.tile([P, NDC, 512], bf16, tag=f"xt{sb}",
                            name=f"xt{sb}") for sb in range(NSB)]
            wv_s = pp.tile([P, NDC, E], bf16, tag="wv")
            wq_s = pp.tile([P, NDC, E], bf16, tag="wq")
            wk_s = pp.tile([P, NDC, E], bf16, tag="wk")
            wo_s = pp.tile([P, NEC, DM], bf16, tag="wo")

            # ---- input DMAs (split across queues so first-needed tensors
            # arrive first: sync carries wv+x, scalar wq/wk, vector wo) ----
            nc.scalar.dma_start(wv_s[:], wv[:])
            for sb in range(NSB):
                nc.sync.dma_start(xt_s[sb][:], xt[sb])
            nc.scalar.dma_start(wq_s[:], wq[:])
            nc.scalar.dma_start(wk_s[:], wk[:])
            nc.scalar.dma_start(wo_s[:], wo[:])
            nc.gpsimd.dma_start(bv_s[:], bvb[:])
            nc.gpsimd.dma_start(bq_s[:], bqt[:])
            nc.gpsimd.dma_start(bk_s[:], bkt[:])
            nc.gpsimd.dma_start(cm_s[:], cm[:])
            nc.gpsimd.dma_start(pb_s[:], pb[:])
            nc.gpsimd.dma_start(ones_s[:], ones[:])

            def v_strip(g):
                sb, ssc = g // 4, g % 4
                psum = psA.tile([P, 512], f32, tag="pa", name=f"pv{g}")
                for dc in range(NDC):
                    nc.tensor.matmul(
                        psum[:],
                        xt_s[sb][:, dc, ssc * P:(ssc + 1) * P],
                        wv_s[:, dc, :],
                        start=(dc == 0), stop=(dc == NDC - 1))
                nc.vector.tensor_tensor(
                    vS[g][:],
                    psum[:].rearrange("p (h d) -> p h d", h=HPC),
                    bv_s[:].rearrange("p (h d) -> p h d", h=HPC),
                    ADD)

            def qk_strip(t, sb):
                for (w_s, b_s, dst, nm) in ((wq_s, bq_s, qT, "q"),
                                            (wk_s, bk_s, kT, "k")):
                    psum = psA.tile([P, 512], f32, tag="pa",
                                    name=f"p{nm}{t}_{sb}")
                    for dc in range(NDC):
                        nc.tensor.matmul(
                            psum[:],
                            w_s[:, dc, t * P:(t + 1) * P],
                            xt_s[sb][:, dc, :],
                            start=(dc == 0), stop=(dc == NDC - 1))
                    nc.vector.tensor_tensor(
                        dst[t][:, sb * 512:(sb + 1) * 512],
                        psum[:],
                        b_s[:, t:t + 1].to_broadcast([P, 512]),
                        ADD)

            def attn_block(t, qb):
                """Head pair (2t, 2t+1), query block qb (512 wide)."""
                hA, hB = 2 * t, 2 * t + 1
                q0 = qb * 512
                nkc = 4 * (qb + 1)
                ops = psO.tile([P, 512], f32, tag="ops", name=f"op{t}_{qb}")
                den = psD.tile([P, 512], f32, tag="den", name=f"dn{t}_{qb}")
                for kc in range(nkc):
                    k0 = kc * P
                    off = max(0, k0 - q0)
                    strip = psS.tile([P, 1024], f32, tag="st",
                                     name=f"st{t}_{qb}_{kc}")
                    # scores^T pair: A rows 0-63, B rows 64-127 (concurrent)
                    nc.tensor.matmul(
                        strip[:, off:512],
                        kT[t][0:DK, k0:k0 + P],
                        qT[t][0:DK, q0 + off:q0 + 512],
                        start=True, stop=True)
                    nc.tensor.matmul(
                        strip[:, 512 + off:1024],
                        kT[t][DK:P, k0:k0 + P],
                        qT[t][DK:P, q0 + off:q0 + 512],
                        start=True, stop=True)
                    exp_s = pw.tile([P, 1024], bf16, tag="exp", bufs=4,
                                    name=f"ex{t}_{qb}_{kc}")
                    if off == 0:
                        nc.scalar.activation(
                            exp_s[:], strip[:], Exp,
                            bias=pb_s[:, kc:kc + 1], scale=SCALE)
                    else:
                        nc.scalar.activation(
                            exp_s[:, off:512], strip[:, off:512], Exp,
                            bias=pb_s[:, kc:kc + 1], scale=SCALE)
                        nc.scalar.activation(
                            exp_s[:, 512 + off:1024],
                            strip[:, 512 + off:1024], Exp,
                            bias=pb_s[:, kc:kc + 1], scale=SCALE)
                    if k0 >= q0:  # diagonal block: causal mask multiply
                        nc.vector.tensor_tensor(
                            exp_s[:, off:off + P],
                            exp_s[:, off:off + P], cm_s[:], MUL)
                        nc.vector.tensor_tensor(
                            exp_s[:, 512 + off:512 + off + P],
                            exp_s[:, 512 + off:512 + off + P], cm_s[:], MUL)
                    first, last = (kc == 0), (kc == nkc - 1)
                    # attn@V pair: A -> rows 0-63, B -> rows 64-127
                    nc.tensor.matmul(
                        ops[0:DK, off:512],
                        vS[kc][:, hA, :], exp_s[:, off:512],
                        start=first, stop=last)
                    nc.tensor.matmul(
                        ops[DK:P, off:512],
                        vS[kc][:, hB, :], exp_s[:, 512 + off:1024],
                        start=first, stop=last)
                    # denominators: M=1 ones-matmuls, rows 0 / 64 (concurrent)
                    nc.tensor.matmul(
                        den[0:1, off:512],
                        ones_s[:], exp_s[:, off:512],
                        start=first, stop=last)
                    nc.tensor.matmul(
                        den[DK:DK + 1, off:512],
                        ones_s[:], exp_s[:, 512 + off:1024],
                        start=first, stop=last)
                # unnormalised out^T straight to SBUF (bf16)
                nc.vector.tensor_copy(
                    outT[t][:, q0:q0 + 512], ops[:])
                # normalise this q-block: den rows -> DRAM -> [128,8] layout
                # for a cheap reciprocal, broadcast back, in-place multiply.
                dcp = pw.tile([DK + 1, 512], f32, tag="dcp", bufs=2,
                              name=f"dc{t}_{qb}")
                nc.vector.tensor_copy(dcp[0:1, :], den[0:1, :])
                nc.vector.tensor_copy(
                    dcp[DK:DK + 1, :], den[DK:DK + 1, :])
                d2 = scr.tile([2, 512], f32, tag="d2", name=f"d2_{t}_{qb}")
                nc.gpsimd.dma_start(d2[:], dcp[0:DK + 1:DK, :])
                dT = pw.tile([P, 8], f32, tag="dT", bufs=2,
                             name=f"dT{t}_{qb}")
                nc.gpsimd.dma_start(
                    dT[:], d2[:].rearrange("r (p j) -> (r p) j", p=DK))
                rcpT = pw.tile([P, 8], f32, tag="rcpT", bufs=2,
                               name=f"rT{t}_{qb}")
                nc.vector.reciprocal(rcpT[:], dT[:])
                d2r = scr.tile([2, 512], f32, tag="d2r", name=f"dr{t}_{qb}")
                nc.gpsimd.dma_start(
                    d2r[:].rearrange("r (p j) -> (r p) j", p=DK), rcpT[:])
                bc = pw.tile([P, 512], f32, tag="bc", bufs=2,
                             name=f"bc{t}_{qb}")
                nc.gpsimd.dma_start(
                    bc[0:DK, :], d2r[0:1, :].to_broadcast([DK, 512]))
                nc.gpsimd.dma_start(
                    bc[DK:P, :], d2r[1:2, :].to_broadcast([DK, 512]))
                nc.vector.tensor_tensor(
                    outT[t][:, q0:q0 + 512],
                    outT[t][:, q0:q0 + 512], bc[:], MUL)

            # ---- interleaved emission: V strips and qk strips woven into
            # the attention stream so the PE never idles ----
            for g in range(4):
                v_strip(g)
            qk_strip(0, 0)
            for t in range(NEC):
                for qb in range(NQB):
                    attn_block(t, qb)
                    if t == 0 and qb < 3:
                        for g in range(4 * (qb + 1), 4 * (qb + 2)):
                            v_strip(g)
                    if (t, qb) < (NEC - 1, NQB - 1):
                        nt, nsb = (t, qb + 1) if qb < 3 else (t + 1, 0)
                        qk_strip(nt, nsb)

            # ---- Phase C: y = out @ Wo^T (bf16 out) ----
            for sc in range(NSC):
                y_s = pw.tile([P, DM], bf16, tag="ys", bufs=2,
                              name=f"ys{sc}")
                for eh in range(2):
                    psum = psA.tile([P, 512], f32, tag="pa",
                                    name=f"py{sc}_{eh}")
                    for dcc in range(NEC):
                        nc.tensor.matmul(
                            psum[:],
                            outT[dcc][:, sc * P:(sc + 1) * P],
                            wo_s[:, dcc, eh * 512:(eh + 1) * 512],
                            start=(dcc == 0), stop=(dcc == NEC - 1))
                    nc.vector.tensor_copy(
                        y_s[:, eh * 512:(eh + 1) * 512], psum[:])
                nc.sync.dma_start(y[sc * P:(sc + 1) * P, :], y_s[:])

    _split_sync_waits(nc, mybir)
    return nc


def _prep_inputs(x, pad_mask, Wq, bq, Wk, bk, Wv, bv, Wo, bo):
    """Build the 8 per-core input maps."""
    import ml_dtypes
    bf16 = ml_dtypes.bfloat16

    def tile3(a, n):  # [n*128, F] -> [128, n, F] in bf16
        return np.ascontiguousarray(
            a.reshape(n, P, a.shape[1]).transpose(1, 0, 2).astype(bf16))

    cmv = (np.arange(P)[:, None] <= np.arange(P)[None, :]).astype(np.float32)
    in_maps = []
    for c in range(NCORES):
        b, tp = c // 2, c % 2
        sl = slice(tp * E, (tp + 1) * E)
        xT = np.ascontiguousarray(x[b].T.astype(np.float32))
        padb = np.where(pad_mask[b, 0, 0] == 1, -1e30, 0.0).astype(np.float32)
        in_maps.append({
            "xt": np.ascontiguousarray(
                tile3(xT, NDC).reshape(P, NDC, NSB, 512)
                .transpose(2, 0, 1, 3)),
            "wq": tile3(np.ascontiguousarray(Wq.T[:, sl]), NDC),
            "wk": tile3(np.ascontiguousarray(Wk.T[:, sl]), NDC),
            "wv": tile3(np.ascontiguousarray(Wv.T[:, sl]), NDC),
            "wo": tile3(np.ascontiguousarray(Wo.T[sl, :]), NEC),
            "bqt": np.ascontiguousarray(bq[sl].reshape(NEC, P).T),
            "bkt": np.ascontiguousarray(bk[sl].reshape(NEC, P).T),
            "bvb": np.ascontiguousarray(np.tile(bv[sl][None, :], (P, 1))),
            "cm": cmv,
            "ones": np.ones((P, 1), dtype=bf16),
            "pb": np.ascontiguousarray(padb.reshape(NSC, P).T),
        })
    return in_maps


def _enable_tracing():
    """Register the NTFF profile hook (the image lacks antenv.axon_hooks)
    and neuter the bucket upload the trace path attempts."""
    import sys
    import types
    try:
        import antenv.axon_hooks  # noqa: F401
    except ImportError:
        from trn_agent_boot.trn_boot import _ntff_profile_via_ctypes
        m = types.ModuleType("antenv.axon_hooks")
        hook = _ntff_profile_via_ctypes("/opt/axon/libaxon_pjrt.so")
        m.get_axon_ntff_profile_hook = lambda: hook
        sys.modules["antenv.axon_hooks"] = m
    import concourse.bass_utils as bu
    bu.upload_artifacts = lambda tmpdir: tmpdir


def _run_once(inputs, trace):
    from concourse.bass_utils import run_bass_kernel_spmd

    if "nc" not in _CACHED:
        _CACHED["nc"] = _build()
    nc = _CACHED["nc"]
    if "in_maps" not in _CACHED or _CACHED.get("in_key") != id(inputs):
        _CACHED["in_maps"] = _prep_inputs(**inputs)
        _CACHED["in_key"] = id(inputs)
    res = run_bass_kernel_spmd(nc, _CACHED["in_maps"],
                               core_ids=list(range(NCORES)), trace=trace)
    bo = inputs["bo"].astype(np.float32)
    out = np.empty((BSZ, SEQ, DM), dtype=np.float32)
    for b in range(BSZ):
        out[b] = (res.results[2 * b]["y"].astype(np.float32)
                  + res.results[2 * b + 1]["y"].astype(np.float32) + bo)
    return out, res


def kernel_with_stats(inputs, trace=False):
    if trace:
        try:
            _enable_tracing()
        except Exception:
            trace = False

    out, res = _run_once(inputs, trace)
    # The first run on a freshly-opened device occasionally returns stale
    # garbage (input upload race in the runtime); retry on insane output.
    for _ in range(2):
        m = np.abs(out).max()
        if np.isfinite(m) and m < 1e3:
            break
        out, res = _run_once(inputs, trace)
    return out, res


def kernel(**inputs):
    out, _ = kernel_with_stats(
        inputs, trace=bool(int(os.environ.get("KERNEL_TRACE", "0"))))
    return out
